# revision 25
# baseline (speedup 1.0000x reference)
"""Trainium2 Bass kernel for DigitConvolutionalModel.

Reference computation (B = 32768):
    x: [B, 784] -> reshape [B, 28, 28]
    conv 3x3 valid with w_conv -> [B, 26, 26] -> [B, 676]
    h1 = relu(conv @ W1 + b1)    W1: [676, 100]
    h2 = relu(h1 @ W2 + b2)      W2: [100, 100]
    out = h2 @ W3 + b3           W3: [100, 10]

Strategy
--------
Pure data parallel: batch split 8 ways (4096 rows/core), weights replicated.
The conv is linear, so it is folded into W1 on the host:
    conv(x) @ W1 == x @ (M @ W1) = x @ W1e,  W1e: [784, 100]
removing the conv from the device entirely (exact up to fp rounding).

On-device layout is "transposed": features on SBUF partitions, batch on the
free dimension, so each layer's PSUM output feeds the next matmul directly
as the moving operand.

x streams in float8 e3m4 (1 byte/elem): the PE multiplies mixed
fp8-moving x fp16-stationary operands (both upcast to fp22 internally), so
only x carries quantization error (~1.5e-2 absmax vs the fp32 reference,
measured host AND hardware; threshold 2e-2) while x HBM traffic halves vs
fp16. Weights stay fp16 and are padded to 128 stationary columns so every
LDWEIGHTS is FWL-eligible (overlapped with the previous matmul): padded
matmuls measure 216 ns/512 cols (2.37 GHz) vs 259 ns unpadded. The
contraction is zero-padded 784 -> 7*128 so all seven chunks are full
128-row matmuls (a separate 16-row tail matmul measured +100 ns each).
The layer-1 bias rides the padded contraction for free: x row 784 is
constant 1.0 (exact in fp8) and W1 row 784 holds b1.

ALL weights+biases ship as ONE packed [128, 1154] fp16 tensor in a single
DMA on the same ring as x, ahead of it. (A separate small-descriptor bias
DMA measured 8+ us of packet-round-robin crawl behind the x stream,
stalling the first epilogue and re-cooling the PE clock.) x is blocked
into eight 512-column blocks, each [128, 7*512] contiguous per partition
-> one DMA with 128 long descriptors per block, all resident in SBUF
(28 KB/partition). Fine blocks keep the PE fed (DMA-completion semaphore
latency is ~1.1-1.25 us, so coarse groups left the PE starving while the
next group landed). Each block's relu/layer-2/layer-3 epilogue is emitted
into the next block's matmul stream; the final block's epilogue runs in
half-width pieces to shorten the exposed pipeline drain.

The PE clock gate needs ~4-5 us of near-continuous PE activity to reach
full rate (216 ns/512-col matmul) and long idle gaps drop it back
(427 ns), so a gapless train of dummy 256-col warmup matmuls starts at
kernel entry, sized to end right as the first block lands.
"""

import numpy as np

N_CORES = 8
B = 32768
B_LOC = B // N_CORES          # 4096 rows per core
NT = 512                      # block width = matmul moving dim = PSUM bank
NB = B_LOC // NT              # 8 column blocks
KC = 7                        # contraction chunks (784 zero-padded to 896)
H = 100                       # hidden width
HP = 128                      # stationary-operand column pad (FWL)
O = 10                        # output width
N_PS1 = 5                     # rotating layer-1 PSUM accumulator banks
WARMUP_MMS = 20               # dummy matmuls to warm the PE clock gate
WNT = 256                     # warmup matmul moving cols
FNT = 256                     # final-epilogue piece width
# packed weight tensor columns: w1 | W2pad | W3pad | b2 | b3
WTC = KC * HP + 2 * HP + 2    # 1154

_COMPILED = {}
LAST_RESULTS = None


def _build_nc():
    import concourse.mybir as mybir
    from concourse import bacc
    from concourse.tile import TileContext

    f32 = mybir.dt.float32
    f16 = mybir.dt.float16
    f8 = mybir.dt.float8e3

    nc = bacc.Bacc(
        "TRN2", target_bir_lowering=False, debug=False, num_devices=N_CORES
    )
    # block b occupies columns [KC*NT*b, KC*NT*(b+1)) with
    # per-partition-contiguous [c, n] layout -> one DMA per block
    xt = nc.dram_tensor("xt", [128, KC * B_LOC], f8, kind="ExternalInput")
    wt = nc.dram_tensor("wt", [128, WTC], f16, kind="ExternalInput")
    ot = nc.dram_tensor("ot", [O, B_LOC], f16, kind="ExternalOutput")

    relu = mybir.ActivationFunctionType.Relu
    add = mybir.AluOpType.add
    amax = mybir.AluOpType.max

    with TileContext(nc) as tc:
        with (
            tc.tile_pool(name="wpool", bufs=1) as wpool,
            tc.tile_pool(name="xpool", bufs=1) as xpool,
            tc.tile_pool(name="hpool", bufs=3) as hpool,
            tc.tile_pool(name="opool", bufs=3) as opool,
            tc.tile_pool(name="ppool", bufs=1, space="PSUM") as ppool,
        ):
            # warmup: zero tile + dummy matmuls with no data dependencies;
            # they run during the initial DMA wait and pull the PE clock
            # out of its idle gate before the first real matmul. gpsimd is
            # otherwise idle at entry; alternating PSUM tags avoids WAW
            # serialization bubbles between consecutive warmups.
            wu_t = wpool.tile([128, WNT], f16)
            nc.gpsimd.memset(wu_t, 0.0)
            for i in range(WARMUP_MMS):
                wu_ps = ppool.tile([128, NT], f32, tag=f"ps{2 + i % 2}",
                                   bufs=2 - i % 2, name=f"wups_{i}")
                nc.tensor.matmul(
                    wu_ps[:, :WNT], lhsT=wu_t[:, 0:HP], rhs=wu_t,
                    start=True, stop=True,
                )

            # everything rides the sync HWDGE ring: packed weights first,
            # then the x blocks (one resident tile + one DMA per block).
            # block 0 is chunk-split (chunks 0-3 | 4-6) into two DMAs so
            # its first matmuls start one partial-transfer earlier, at
            # full matmul width. the last 512 columns are split into two
            # 256-wide blocks so the final exposed epilogue is half-size.
            wt_t = wpool.tile([128, WTC], f16)
            nc.sync.dma_start(out=wt_t, in_=wt.ap())
            NH = NT // 2
            xb0a = xpool.tile([128, 4 * NT], f8, name="xb0a")
            nc.sync.dma_start(out=xb0a, in_=xt.ap()[:, 0 : 4 * NT])
            xb0b = xpool.tile([128, 3 * NT], f8, name="xb0b")
            nc.sync.dma_start(out=xb0b, in_=xt.ap()[:, 4 * NT : KC * NT])
            xb_ts = []
            for b in range(1, NB - 1):
                xb = xpool.tile([128, KC * NT], f8, name=f"xb{b}")
                nc.sync.dma_start(
                    out=xb, in_=xt.ap()[:, KC * NT * b : KC * NT * (b + 1)]
                )
                xb_ts.append(xb)
            base = KC * NT * (NB - 1)
            x7_ts = []
            for hh in range(2):
                xh = xpool.tile([128, KC * NH], f8, name=f"xb7{'ab'[hh]}")
                nc.sync.dma_start(
                    out=xh,
                    in_=xt.ap()[:, base + KC * NH * hh : base + KC * NH * (hh + 1)],
                )
                x7_ts.append(xh)

            w2_t = wt_t[:H, KC * HP : KC * HP + HP]
            w3_t = wt_t[:H, KC * HP + HP : KC * HP + 2 * HP]
            # DVE tensor_scalar needs f32 scalars: widen the packed fp16
            # biases once on (otherwise idle) gpsimd
            b23_t = wpool.tile([H, 2], f32)
            nc.gpsimd.tensor_scalar_add(b23_t, wt_t[:H, WTC - 2 : WTC], 0.0)
            b2_t = b23_t[:H, 0:1]
            b3_t = b23_t[:O, 1:2]

            def epilogue(n_off, width, ps1, pw=NT):
                # pw < width splits the block into stage-major pieces for
                # a shorter exposed pipeline drain
                h1s, h2s, o_ts = [], [], []
                pieces = [(p0, min(pw, width - p0))
                          for p0 in range(0, width, pw)]
                for i, (p0, w) in enumerate(pieces):
                    h1 = hpool.tile([H, w], f16, tag="h1", bufs=4,
                                    name=f"h1_{i}")
                    nc.scalar.activation(h1, ps1[:H, p0 : p0 + w], relu)
                    h1s.append(h1)
                for i, (p0, w) in enumerate(pieces):
                    ps2 = ppool.tile([128, NT], f32, tag="ps2", bufs=2,
                                     name="ps2")
                    nc.tensor.matmul(
                        ps2[:, :w], lhsT=w2_t, rhs=h1s[i],
                        start=True, stop=True,
                    )
                    h2 = hpool.tile([H, w], f16, tag="h2", bufs=4,
                                    name=f"h2_{i}")
                    nc.vector.tensor_scalar(h2, ps2[:H, :w], b2_t, 0.0,
                                            add, amax)
                    h2s.append(h2)
                for i, (p0, w) in enumerate(pieces):
                    ps3 = ppool.tile([128, NT], f32, tag="ps3", bufs=1,
                                     name="ps3")
                    nc.tensor.matmul(
                        ps3[:, :w], lhsT=w3_t, rhs=h2s[i],
                        start=True, stop=True,
                    )
                    o_t = opool.tile([O, w], f16, tag="o_t", bufs=4,
                                     name=f"o_{i}")
                    nc.scalar.add(o_t, ps3[:O, :w], b3_t)
                    o_ts.append(o_t)
                # out-DMAs on gpsimd (idle): keeps DMA issue off the ACT
                # queue, which head-of-line blocks the relu/add chain
                for i, (p0, w) in enumerate(pieces):
                    n0 = n_off + p0
                    nc.gpsimd.dma_start(
                        out=ot.ap()[:, n0 : n0 + w], in_=o_ts[i]
                    )

            # work items: (rhs-for-chunk-c, width, output col offset, tag)
            def b0_rhs(c):
                if c < 4:
                    return xb0a[:, c * NT : (c + 1) * NT]
                return xb0b[:, (c - 4) * NT : (c - 3) * NT]

            items = [(b0_rhs, NT, 0, "ps1_0")]
            for b in range(1, NB - 1):
                items.append((
                    (lambda xb: lambda c: xb[:, c * NT : (c + 1) * NT])(
                        xb_ts[b - 1]
                    ),
                    NT, b * NT, f"ps1_{b % N_PS1}",
                ))
            for hh in range(2):
                items.append((
                    (lambda xh: lambda c: xh[:, c * NH : (c + 1) * NH])(
                        x7_ts[hh]
                    ),
                    NH, (NB - 1) * NT + hh * NH, f"ps1_{(2 + hh)}",
                ))

            # software pipeline: each item's epilogue is emitted one chunk
            # into the next item's mm1 stream
            pending = None  # (n_off, width, ps1)
            for i, (rhs_fn, width, n_off, tag) in enumerate(items):
                ps1 = ppool.tile([128, NT], f32, tag=tag, bufs=1,
                                 name=f"ps1_{i}")
                for c in range(KC):
                    nc.tensor.matmul(
                        ps1[:, :width],
                        lhsT=wt_t[:, c * HP : (c + 1) * HP],
                        rhs=rhs_fn(c),
                        start=(c == 0),
                        stop=(c == KC - 1),
                    )
                    if c == 0 and pending is not None:
                        epilogue(*pending)
                        pending = None
                pending = (n_off, width, ps1)
            epilogue(*pending, pw=FNT)

    nc.finalize()
    return nc


def _fold_conv_into_w1(w_conv, W1):
    """W1e[784, 100] such that x @ W1e == conv3x3(x) @ W1 (exact fold)."""
    W1e = np.zeros((28, 28, H), np.float64)
    W1r = W1.astype(np.float64).reshape(26, 26, H)
    wc = w_conv.astype(np.float64)
    for di in range(3):
        for dj in range(3):
            W1e[di : di + 26, dj : dj + 26, :] += wc[di, dj] * W1r
    return W1e.reshape(784, H).astype(np.float32)


def kernel(x, w_conv, W1, b1, W2, b2, W3, b3):
    import ml_dtypes
    from concourse.bass_utils import run_bass_kernel_spmd

    global LAST_RESULTS

    f8 = ml_dtypes.float8_e3m4

    x = np.asarray(x, np.float32)
    W1e = _fold_conv_into_w1(np.asarray(w_conv), np.asarray(W1))
    # augmented layer-1 weights: rows 0..783 = W1e, row 784 = b1 (the
    # matching x row is constant 1.0), rows 785.. = 0
    W1e_pad = np.zeros((KC * 128, H), np.float32)
    W1e_pad[:784] = W1e
    W1e_pad[784] = np.asarray(b1, np.float32)
    # packed weight tensor [128, WTC]: per-partition p, cols c*128+j hold
    # W1e_pad[c*128+p, j]; then W2 | W3 | b2 | b3 (partitions 0..H-1)
    wt_dev = np.zeros((128, WTC), np.float16)
    w1_blk = W1e_pad.reshape(KC, 128, H).transpose(1, 0, 2)  # [128, KC, H]
    for c in range(KC):
        wt_dev[:, c * HP : c * HP + H] = w1_blk[:, c, :]
    wt_dev[:H, KC * HP : KC * HP + H] = np.asarray(W2, np.float32).astype(
        np.float16
    )
    wt_dev[:H, KC * HP + HP : KC * HP + HP + O] = np.asarray(
        W3, np.float32
    ).astype(np.float16)
    wt_dev[:H, WTC - 2] = np.asarray(b2, np.float32).astype(np.float16)
    wt_dev[:O, WTC - 1] = np.asarray(b3, np.float32).astype(np.float16)

    in_maps = []
    for c in range(N_CORES):
        xs = x[c * B_LOC : (c + 1) * B_LOC]          # [B_LOC, 784]
        xT = np.zeros((KC * 128, B_LOC), f8)          # zero-padded features
        xT[:784] = xs.T.astype(f8)                    # [896, B_LOC] fp8
        xT[784] = f8(1.0)                             # bias row
        # block-contiguous [128, KC*B_LOC]: each block [128, KC, width]
        # flattened per partition; the last 512 cols split into 2 halves
        xmain = xT.reshape(KC, 128, B_LOC)
        widths = [NT] * (NB - 1) + [NT // 2, NT // 2]
        blocks = []
        g0 = 0
        for w in widths:
            blocks.append(
                xmain[:, :, g0 : g0 + w].transpose(1, 0, 2).reshape(
                    128, KC * w
                )
            )
            g0 += w
        xt_dev = np.ascontiguousarray(np.concatenate(blocks, axis=1))
        in_maps.append({"xt": xt_dev, "wt": wt_dev})

    if "nc" not in _COMPILED:
        _COMPILED["nc"] = _build_nc()
    nc = _COMPILED["nc"]

    res = run_bass_kernel_spmd(nc, in_maps, core_ids=list(range(N_CORES)))
    LAST_RESULTS = res

    out = np.empty((B, O), np.float32)
    for c in range(N_CORES):
        out[c * B_LOC : (c + 1) * B_LOC] = res.results[c]["ot"].T
    return out
